# revision 19
# baseline (speedup 1.0000x reference)
"""Trainium2 Bass kernel for the gnn_message_passing DepthWise block.

Computation (see problem reference):
    h   = x @ W1 + b1                      # [N, G]
    h   = LayerNorm(h) * ln_g + ln_b       # over channels, eps=1e-6
    acc = sum_k h[idx[:, k]] * dw_w[k]     # depthwise gather conv, K=27
    h2  = (acc + dw_b) @ W2 + b2           # [N, C_OUT]
    g   = gelu(h2)                          # exact erf form
    GRN + residual:
        Gx = ||g||_2 over rows per channel; Nx = Gx / (mean(Gx) + eps)
        out = grn_g * (g * Nx) + grn_b + g + x

v2 design notes (vs the baseline):
  - ln_g/ln_b are folded into dw_w/dw_b on the host, so the feature table
    stores the raw normalized (h - mu) * rstd in fp8e4 (range ~N(0,1)),
    halving both the AllGather and the gather DMA traffic.
  - hsh is split into 8 chunk tensors so 8 chunked AllGathers overlap
    phase-1 compute; the table uses a chunk-major row layout and the
    host remaps neighbor indices to match.
  - x is staged pre-transposed for phase 1 (no DMA transpose) and the
    LN normalize runs on the scalar engine (scale/bias APs), keeping
    phase 1 short so the gather stream starts early.
  - The gather stays 27x one-offset-per-partition indirect DMAs per
    tile: the HW DGE's multi-offset forms mis-pair offsets with dest
    slots (canary-verified), so the SWDGE fixed cost (~1.1us/op) is the
    kernel's critical path; everything else is sized to hide under it.
  - No pad-row masking: the 736 pad rows perturb only the GRN stats by
    ~7e-4 relative, far below the accuracy budget.
"""

import numpy as np

from concourse import bacc, bass, mybir, tile
from concourse.bass_utils import run_bass_kernel_spmd

# ---------------------------------------------------------------- geometry
P = 128
N_CORES = 8
N = 500000
C_IN = 128
G = 256
C_OUT = 128
K = 27
FD = K * G
EPS_LN = 1e-6
EPS_GRN = 1e-6
N_CHUNKS = 8          # AllGather chunks
FT = 8                # tiles per final-pass iteration

BF16 = mybir.dt.bfloat16
F32 = mybir.dt.float32
FP8 = mybir.dt.float8e4
I32 = mybir.dt.int32
NP_BF16 = mybir.dt.np(BF16)
TBL_DT = FP8          # table dtype knob (FP8 or BF16)

ADD = mybir.AluOpType.add
SUB = mybir.AluOpType.subtract
MULT = mybir.AluOpType.mult
BYPASS = mybir.AluOpType.bypass
AF = mybir.ActivationFunctionType


def cfg_tiles(n_pad):
    rpc = n_pad // N_CORES
    assert rpc % P == 0
    return rpc, rpc // P


def pad_size(n):
    q = N_CORES * P
    return (n + q - 1) // q * q


N_PAD = pad_size(N)          # 500736
RPC, T = cfg_tiles(N_PAD)    # 62592 rows/core, 489 tiles/core


def chunk_tile_counts(n_tiles=T, n_chunks=N_CHUNKS):
    base = (n_tiles + n_chunks - 1) // n_chunks
    counts = []
    left = n_tiles
    for _ in range(n_chunks):
        c = min(base, left)
        counts.append(c)
        left -= c
    assert sum(counts) == n_tiles and all(c > 0 for c in counts)
    return counts


CHUNK_TILES = chunk_tile_counts()              # e.g. [62]*7 + [55]
CHUNK_ROWS = [c * P for c in CHUNK_TILES]
# table chunk base row (chunk-major layout: chunk j holds all 8 cores' rows)
TBL_CHUNK_BASE = np.concatenate(
    [[0], np.cumsum([N_CORES * r for r in CHUNK_ROWS])]
).astype(np.int64)
# local-row -> (chunk, within-chunk-pos) lookup pieces
LOCAL_CHUNK_BASE = np.concatenate([[0], np.cumsum(CHUNK_ROWS)]).astype(np.int64)


def remap_rows(r):
    """Map global row ids to chunk-major table row ids (vectorized)."""
    c = r // RPC
    l = r % RPC
    j = np.searchsorted(LOCAL_CHUNK_BASE, l, side="right") - 1
    pos = l - LOCAL_CHUNK_BASE[j]
    rows_j = np.asarray(CHUNK_ROWS, dtype=np.int64)[j]
    return TBL_CHUNK_BASE[j] + c * rows_j + pos


# ---------------------------------------------------------------- program
def build_nc(n_pad=N_PAD, n_cores=N_CORES, gelu_func=None):
    gelu_func = AF.Gelu if gelu_func is None else gelu_func
    rpc, n_tiles = cfg_tiles(n_pad)
    rg = [list(range(n_cores))]

    nc = bacc.Bacc(
        "TRN2", target_bir_lowering=False, debug=False, num_devices=n_cores
    )

    # ---- per-core inputs
    xbfT = nc.dram_tensor("xbfT", [C_IN, rpc], BF16, kind="ExternalInput")
    xrbT = nc.dram_tensor("xrbT", [C_OUT, rpc], F32, kind="ExternalInput")
    idx = nc.dram_tensor("idx", [rpc, K], I32, kind="ExternalInput")
    # ---- replicated weights / constants
    w1 = nc.dram_tensor("w1", [C_IN, G], BF16, kind="ExternalInput")
    b1 = nc.dram_tensor("b1", [1, G], BF16, kind="ExternalInput")
    wb = nc.dram_tensor("wb", [P, FD], BF16, kind="ExternalInput")
    w2 = nc.dram_tensor("w2", [G, C_OUT], BF16, kind="ExternalInput")
    b2p = nc.dram_tensor("b2p", [C_OUT, 1], F32, kind="ExternalInput")
    grngc = nc.dram_tensor("grngc", [C_OUT, 1], F32, kind="ExternalInput")
    identb = nc.dram_tensor("identb", [P, P], BF16, kind="ExternalInput")
    onesb = nc.dram_tensor("onesb", [1, P], BF16, kind="ExternalInput")
    onescf = nc.dram_tensor("onescf", [P, 1], F32, kind="ExternalInput")
    onesrf = nc.dram_tensor("onesrf", [1, P], F32, kind="ExternalInput")
    epsc = nc.dram_tensor("epsc", [P, 2], F32, kind="ExternalInput")
    # ---- internal DRAM
    hshs = [
        nc.dram_tensor(f"hsh{j}", [CHUNK_ROWS[j], G], TBL_DT)
        for j in range(N_CHUNKS)
    ]
    table = nc.dram_tensor("table", [n_pad, G], TBL_DT, addr_space="Shared")
    gel = nc.dram_tensor("gel", [C_OUT, rpc], BF16)
    psq_in = nc.dram_tensor("psq_in", [C_OUT, 1], F32)
    psq_out = nc.dram_tensor("psq_out", [C_OUT, 1], F32, addr_space="Shared")
    # ---- output (transposed layout; host transposes back)
    outT = nc.dram_tensor("outT", [C_OUT, rpc], F32, kind="ExternalOutput")

    with tile.TileContext(nc) as tc:
        with (
            tc.tile_pool(name="const", bufs=1) as cp,
            tc.tile_pool(name="work", bufs=3) as wp,
            tc.tile_pool(name="pref", bufs=8) as fp,
            tc.tile_pool(name="gat", bufs=3) as gp,
            tc.tile_pool(name="mul", bufs=2) as mp,
            tc.tile_pool(name="psum", bufs=2, space="PSUM") as pp,
        ):
            # ---------------- load constants into SBUF
            def cload(dram, shape, dtype, tag):
                t = cp.tile(shape, dtype, tag=tag)
                nc.sync.dma_start(out=t[:], in_=dram[:])
                return t

            w1_s = cload(w1, [C_IN, G], BF16, "w1")
            b1_s = cload(b1, [1, G], BF16, "b1")
            wb_s = cload(wb, [P, FD], BF16, "wb")
            w2_a = cp.tile([P, C_OUT], BF16, tag="w2a")
            nc.sync.dma_start(out=w2_a[:], in_=w2[0:P, :])
            w2_b = cp.tile([P, C_OUT], BF16, tag="w2b")
            nc.sync.dma_start(out=w2_b[:], in_=w2[P:G, :])
            b2p_s = cload(b2p, [C_OUT, 1], F32, "b2p")
            grngc_s = cload(grngc, [C_OUT, 1], F32, "grngc")
            ident_s = cload(identb, [P, P], BF16, "identb")
            ones_s = cload(onesb, [1, P], BF16, "onesb")
            onescf_s = cload(onescf, [P, 1], F32, "onescf")
            onesrf_s = cload(onesrf, [1, P], F32, "onesrf")
            epsc_s = cload(epsc, [P, 2], F32, "epsc")
            psq_all = cp.tile([C_OUT, n_tiles], F32, tag="psqall")

            # ---------------- phase 1: normalized table for own shard
            chunk_of_tile = []
            for j, ct in enumerate(CHUNK_TILES):
                chunk_of_tile += [j] * ct

            PREF = 6

            def load_xT(tt):
                s = fp.tile([C_IN, P], BF16, tag="xT")
                nc.sync.dma_start(out=s[:], in_=xbfT[:, tt * P : tt * P + P])
                return s

            def stage_a(t):
                xT = xT_tiles.pop(t)
                hp = pp.tile([P, G], F32, tag="hp")
                nc.tensor.matmul(
                    out=hp[:], lhsT=ones_s[:], rhs=b1_s[:],
                    start=True, stop=False, skip_group_check=True,
                )
                nc.tensor.matmul(
                    out=hp[:], lhsT=xT[:], rhs=w1_s[:],
                    start=False, stop=True, skip_group_check=True,
                )
                stats6 = wp.tile([P, 6], F32, tag="stats6")
                nc.vector.bn_stats(out=stats6[:], in_=hp[:])
                stats2 = wp.tile([P, 2], F32, tag="stats2")
                nc.vector.bn_aggr(out=stats2[:], in_=stats6[:])
                return hp, stats2

            def stage_b(t, hp, stats2):
                r0 = t * P
                j = chunk_of_tile[t]
                lr0 = r0 - int(LOCAL_CHUNK_BASE[j])
                sd = wp.tile([P, 1], F32, tag="sd")
                nc.scalar.activation(
                    out=sd[:], in_=stats2[:, 1:2], func=AF.Sqrt,
                    bias=epsc_s[:, 0:1],
                )
                rstd = wp.tile([P, 1], F32, tag="rstd")
                nc.vector.reciprocal(out=rstd[:], in_=sd[:])
                nmr = wp.tile([P, 1], F32, tag="nmr")
                nc.vector.tensor_scalar(
                    out=nmr[:], in0=stats2[:, 0:1], scalar1=rstd[:, 0:1],
                    scalar2=-1.0, op0=MULT, op1=MULT,
                )
                hln = wp.tile([P, G], TBL_DT, tag="hln")
                nc.scalar.activation(
                    out=hln[:], in_=hp[:], func=AF.Identity,
                    scale=rstd[:, 0:1], bias=nmr[:, 0:1],
                )
                nc.scalar.dma_start(
                    out=hshs[j][lr0 : lr0 + P, :], in_=hln[:]
                )

            # software-pipelined: stats of tile t+1 issue before the
            # normalize of tile t, so neither engine queue head-blocks.
            xT_tiles = {t: load_xT(t) for t in range(min(PREF, n_tiles))}
            pend = None
            for t in range(n_tiles):
                if t + PREF < n_tiles:
                    xT_tiles[t + PREF] = load_xT(t + PREF)
                cur = (t, *stage_a(t))
                if pend is not None:
                    stage_b(*pend)
                pend = cur
            stage_b(*pend)

            # ---------------- chunked all-gather of the feature table
            for j in range(N_CHUNKS):
                tb0 = int(TBL_CHUNK_BASE[j])
                tb1 = int(TBL_CHUNK_BASE[j + 1])
                nc.gpsimd.collective_compute(
                    "AllGather",
                    BYPASS,
                    replica_groups=rg,
                    ins=[hshs[j].ap().opt()],
                    outs=[table[tb0:tb1, :].opt()],
                )

            # ---------------- phase 3: gather + depthwise + W2 + gelu
            def load_idx(tt):
                s = fp.tile([P, K], I32, tag="idx")
                nc.sync.dma_start(out=s[:], in_=idx[tt * P : tt * P + P, :])
                return s

            idx_tiles = {t: load_idx(t) for t in range(min(PREF, n_tiles))}
            for t in range(n_tiles):
                r0 = t * P
                if t + PREF < n_tiles:
                    idx_tiles[t + PREF] = load_idx(t + PREF)
                idx_s = idx_tiles.pop(t)
                g_t = gp.tile([P, FD], TBL_DT, tag="g")
                g3v = g_t[:].rearrange("p (k c) -> p k c", k=K)
                # One indirect DMA per tap: the only offset-AP form the HW
                # DGE implements correctly is one offset per partition with
                # a contiguous per-partition block ([P,1] offsets, [P,D]
                # dest). Multi-offset forms mis-pair offsets with dest
                # slots (verified with canary probes).
                for k in range(K):
                    nc.gpsimd.indirect_dma_start(
                        out=g3v[:, k, :],
                        out_offset=None,
                        in_=table[:, :],
                        in_offset=bass.IndirectOffsetOnAxis(
                            ap=idx_s[:, k : k + 1], axis=0
                        ),
                    )
                # Depthwise multiply in tap-chunks: finer consumer granularity
                # keeps the semaphore-recycle edges from stalling the gather
                # stream on a whole-tile multiply.
                gm = mp.tile([P, FD], BF16, tag="gm")
                for a, b in ((0, 7), (7, 14), (14, 21), (21, K)):
                    nc.vector.tensor_tensor(
                        out=gm[:, a * G : b * G],
                        in0=g_t[:, a * G : b * G],
                        in1=wb_s[:, a * G : b * G],
                        op=MULT,
                    )
                # k-sum via accumulating identity matmuls
                acc = pp.tile([P, G], F32, tag="acc")
                g3 = gm[:].rearrange("p (k g) -> p k g", k=K)
                for k in range(K):
                    nc.tensor.matmul(
                        out=acc[:], lhsT=ident_s[:], rhs=g3[:, k, :],
                        start=(k == 0), stop=(k == K - 1),
                    )
                acc_sb = wp.tile([P, G], BF16, tag="accsb")
                nc.scalar.copy(out=acc_sb[:], in_=acc[:])
                accT = pp.tile([P, 2, P], BF16, tag="accT")
                nc.tensor.transpose(
                    out=accT[:, 0, :], in_=acc_sb[:, 0:P], identity=ident_s[:]
                )
                nc.tensor.transpose(
                    out=accT[:, 1, :], in_=acc_sb[:, P:G], identity=ident_s[:]
                )
                accT_sb = wp.tile([P, 2, P], BF16, tag="accTsb")
                nc.scalar.copy(out=accT_sb[:, 0, :], in_=accT[:, 0, :])
                nc.scalar.copy(out=accT_sb[:, 1, :], in_=accT[:, 1, :])
                o2 = pp.tile([C_OUT, P], F32, tag="o2", bufs=1)
                nc.tensor.matmul(
                    out=o2[:], lhsT=w2_a[:], rhs=accT_sb[:, 0, :],
                    start=True, stop=False,
                )
                nc.tensor.matmul(
                    out=o2[:], lhsT=w2_b[:], rhs=accT_sb[:, 1, :],
                    start=False, stop=True,
                )
                gt = wp.tile([C_OUT, P], BF16, tag="gt")
                nc.scalar.activation(
                    out=gt[:], in_=o2[:], func=gelu_func, bias=b2p_s[:]
                )
                sq = wp.tile([C_OUT, P], BF16, tag="sq")
                nc.scalar.activation(
                    out=sq[:], in_=gt[:], func=AF.Square,
                    accum_out=psq_all[:, t : t + 1],
                )
                nc.scalar.dma_start(out=gel[:, r0 : r0 + P], in_=gt[:])

            # ---------------- GRN stats: reduce + all-reduce + scale
            psq_col = wp.tile([C_OUT, 1], F32, tag="psqcol")
            nc.vector.tensor_reduce(
                out=psq_col[:], in_=psq_all[:], axis=mybir.AxisListType.X, op=ADD
            )
            nc.sync.dma_start(out=psq_in[:, :], in_=psq_col[:])
            nc.gpsimd.collective_compute(
                "AllReduce",
                ADD,
                replica_groups=rg,
                ins=[psq_in.ap().opt()],
                outs=[psq_out.ap().opt()],
            )
            ssq = wp.tile([C_OUT, 1], F32, tag="ssq")
            nc.sync.dma_start(out=ssq[:], in_=psq_out[:, :])
            gx = wp.tile([C_OUT, 1], F32, tag="gx")
            nc.scalar.activation(out=gx[:], in_=ssq[:], func=AF.Sqrt, bias=0.0)
            smean = pp.tile([1, 1], F32, tag="small", bufs=1, name="smean")
            nc.tensor.matmul(
                out=smean[:], lhsT=onescf_s[:], rhs=gx[:], start=True, stop=True
            )
            s0 = wp.tile([1, 1], F32, tag="s0")
            nc.scalar.activation(
                out=s0[:], in_=smean[:], func=AF.Identity,
                bias=epsc_s[0:1, 1:2], scale=1.0 / C_OUT,
            )
            rec = wp.tile([1, 1], F32, tag="rec")
            nc.vector.reciprocal(out=rec[:], in_=s0[:])
            recb = pp.tile([C_OUT, 1], F32, tag="small", bufs=1, name="recb")
            nc.tensor.matmul(
                out=recb[:], lhsT=onesrf_s[:], rhs=rec[:], start=True, stop=True
            )
            nx = wp.tile([C_OUT, 1], F32, tag="nx")
            nc.vector.tensor_tensor(out=nx[:], in0=recb[:], in1=gx[:], op=MULT)
            ga = wp.tile([C_OUT, 1], F32, tag="ga")
            nc.vector.tensor_tensor(out=ga[:], in0=nx[:], in1=grngc_s[:], op=MULT)
            a2 = wp.tile([C_OUT, 1], F32, tag="a2")
            nc.scalar.activation(out=a2[:], in_=ga[:], func=AF.Identity, bias=1.0)

            # ---------------- final: out = a2 (.) gelu + (x + grn_b)
            fchunks = list(range(0, n_tiles, FT))

            def load_fin(t0):
                r0 = t0 * P
                w = min(FT, n_tiles - t0) * P
                g2 = wp.tile([C_OUT, FT * P], BF16, tag="gt2")
                nc.sync.dma_start(out=g2[:, :w], in_=gel[:, r0 : r0 + w])
                xw = wp.tile([C_OUT, FT * P], F32, tag="xt")
                nc.sync.dma_start(out=xw[:, :w], in_=xrbT[:, r0 : r0 + w])
                return g2, xw

            FPREF = 2
            fin_tiles = {c: load_fin(fchunks[c])
                         for c in range(min(FPREF, len(fchunks)))}
            for c, t0 in enumerate(fchunks):
                r0 = t0 * P
                w = min(FT, n_tiles - t0) * P
                if c + FPREF < len(fchunks):
                    fin_tiles[c + FPREF] = load_fin(fchunks[c + FPREF])
                gt2, xt = fin_tiles.pop(c)
                u = wp.tile([C_OUT, FT * P], F32, tag="u")
                nc.scalar.mul(out=u[:, :w], in_=gt2[:, :w], mul=a2[:])
                ot = wp.tile([C_OUT, FT * P], F32, tag="ot")
                nc.vector.tensor_tensor(
                    out=ot[:, :w], in0=u[:, :w], in1=xt[:, :w], op=ADD
                )
                nc.scalar.dma_start(out=outT[:, r0 : r0 + w], in_=ot[:, :w])

    nc.compile()
    return nc


# ---------------------------------------------------------------- host side
def _prep_inputs(x, neighbor_idx, W1, b1, ln_g, ln_b, dw_w, dw_b, W2, b2,
                 grn_g, grn_b, n_pad=N_PAD, n_cores=N_CORES):
    rpc, n_tiles = cfg_tiles(n_pad)
    n = x.shape[0]

    xp = np.zeros((n_pad, C_IN), np.float32)
    xp[:n] = x
    idxp = np.zeros((n_pad, K), np.int64)
    idxp[:n] = neighbor_idx
    # remap to chunk-major table rows
    idxr = remap_rows(idxp.ravel()).reshape(n_pad, K).astype(np.int32)



    xbf = xp.astype(NP_BF16)
    xrb = xp + grn_b.reshape(1, C_OUT).astype(np.float32)

    ln_g64 = ln_g.astype(np.float64).reshape(1, G)
    ln_b64 = ln_b.astype(np.float64).reshape(G)
    dw64 = dw_w.astype(np.float64)
    w1b = W1.astype(NP_BF16)
    b1b = b1.reshape(1, G).astype(NP_BF16)
    # fold ln_g into the depthwise weights, ln_b into the dw bias
    wbf = np.broadcast_to(
        (dw64 * ln_g64).astype(np.float32).reshape(1, FD), (P, FD)
    ).astype(NP_BF16).copy()
    dwb_eff = dw_b.astype(np.float64) + ln_b64 * dw64.sum(axis=0)
    w2b = W2.astype(NP_BF16)
    b2p = (dwb_eff @ W2.astype(np.float64)
           + b2.astype(np.float64)).astype(np.float32).reshape(C_OUT, 1)
    grngc = grn_g.reshape(C_OUT, 1).astype(np.float32)
    identb = np.eye(P, dtype=NP_BF16)
    onesb = np.ones((1, P), NP_BF16)
    onescf = np.ones((P, 1), np.float32)
    onesrf = np.ones((1, P), np.float32)
    epsc_arr = np.broadcast_to(
        np.array([[EPS_LN, EPS_GRN]], np.float32), (P, 2)
    ).copy()

    in_maps = []
    for c in range(n_cores):
        r0 = c * rpc
        sl = slice(r0, r0 + rpc)
        in_maps.append({
            "xbfT": np.ascontiguousarray(xbf[sl].T),
            "xrbT": np.ascontiguousarray(xrb[sl].T),
            "idx": np.ascontiguousarray(idxr[sl]),
            "w1": w1b, "b1": b1b,
            "wb": wbf, "w2": w2b, "b2p": b2p, "grngc": grngc,
            "identb": identb, "onesb": onesb,
            "onescf": onescf, "onesrf": onesrf, "epsc": epsc_arr,
        })
    return in_maps


_NC_CACHE = {}


def _get_nc(n_pad=N_PAD, n_cores=N_CORES):
    key = (n_pad, n_cores)
    if key not in _NC_CACHE:
        _NC_CACHE[key] = build_nc(n_pad, n_cores)
    return _NC_CACHE[key]


def kernel(x, neighbor_idx, W1, b1, ln_g, ln_b, dw_w, dw_b, W2, b2,
           grn_g, grn_b, _trace=False, _trace_cores=None):
    x = np.asarray(x, np.float32)
    neighbor_idx = np.asarray(neighbor_idx, np.int32)
    args = [np.asarray(a) for a in
            (W1, b1, ln_g, ln_b, dw_w, dw_b, W2, b2, grn_g, grn_b)]

    nc = _get_nc()
    in_maps = _prep_inputs(x, neighbor_idx, *args)
    res = run_bass_kernel_spmd(
        nc, in_maps, core_ids=list(range(N_CORES)),
        trace=_trace, trace_cores=_trace_cores,
    )
    n = x.shape[0]
    rpc, _ = cfg_tiles(N_PAD)
    out = np.empty((N_PAD, C_OUT), np.float32)
    for c in range(N_CORES):
        out[c * rpc : (c + 1) * rpc] = res.results[c]["outT"].T
    if _trace:
        kernel._last_result = res
    return out[:n]
